# revision 25
# baseline (speedup 1.0000x reference)
"""Trainium2 Bass kernel for AttentionSecondOrderRNN.

Strategy: time-shard the T=2048 scan across 8 cores. Each core runs a
contiguous 256-step chunk plus a 64-step warmup (the recurrence contracts
~0.4x/step through tanh, so the state converges from h=0 well within 64
steps). Core 0's warmup is masked (alpha forced to 0) so its state stays
exactly 0 until its chunk begins. Full batch B=64 lives on every core.

Per-core layout (H=256, K=4, E=256, B=64):
  - contraction chunks c in {0,1}: h/e dims [128c, 128c+128)
  - output col-groups f in {0,1}: g-half [128f, 128f+128), PE col-tiled
  - PSUM cell tile [128, 512]: partition = 64f + b, free = k*128 + g''
  - per step: PE mms compute bias + x.Wih + h.Whh into one PSUM bank,
    ACT applies tanh, DVE does the alpha-weighted k-sum, PE transposes
    h_new back to [h, b] for the next step's stationary operand.
"""

import os

import numpy as np
import ml_dtypes

import concourse.bass as bass
import concourse.mybir as mybir
import concourse.tile as tile
from concourse.bass_utils import run_bass_kernel_spmd

BF16 = mybir.dt.bfloat16
F32 = mybir.dt.float32

B, T, E, H, K = 64, 2048, 256, 256, 4
NCORES = 8
W = 64                      # warmup steps
TCHUNK = T // NCORES        # 256
TC = TCHUNK + W             # 320 local steps per core
PIPE = 3                    # xW precompute pipeline depth (PSUM banks)


def build_bass():
    nc = bass.Bass()

    # ---- DRAM inputs (host-marshaled layouts) ----
    x_t = nc.dram_tensor("x_t", [128, TC * 128], BF16, kind="ExternalInput")
    whh_mov = nc.dram_tensor("whh_mov", [128, 2048], BF16, kind="ExternalInput")
    wih_mov = nc.dram_tensor("wih_mov", [128, 2048], BF16, kind="ExternalInput")
    # brow = ones[0:128] ++ bias_row[128:1152] in one tensor (one DMA, one sem)
    brow = nc.dram_tensor("brow", [1, 1152], BF16, kind="ExternalInput")
    v_mov = nc.dram_tensor("v_mov", [128, 8], BF16, kind="ExternalInput")
    ident = nc.dram_tensor("ident", [128, 128], F32, kind="ExternalInput")
    m_mask = nc.dram_tensor("m_mask", [64, TC], F32, kind="ExternalInput")

    y_out = nc.dram_tensor("y_out", [128, TCHUNK * 128], F32, kind="ExternalOutput")
    YB = 32  # y staging block (steps per out-DMA)

    with tile.TileContext(nc) as tc:
        with (
            tc.tile_pool(name="const", bufs=1) as cpool,
            tc.tile_pool(name="alpha", bufs=1) as apool,
            tc.tile_pool(name="state", bufs=2) as spool,
            tc.tile_pool(name="work", bufs=3) as wpool,
            tc.tile_pool(name="ystage", bufs=2) as ypool,
            tc.tile_pool(name="cells", bufs=PIPE + 1, space="PSUM") as cells_pool,
            tc.tile_pool(name="misc_psum", bufs=2, space="PSUM") as mpsum,
        ):
            # ---- load constants into SBUF ----
            x_sb = cpool.tile([128, TC * 128], BF16, tag="x")
            nc.sync.dma_start(out=x_sb[:], in_=x_t[:])
            whh_sb = cpool.tile([128, 2048], BF16, tag="whh")
            nc.sync.dma_start(out=whh_sb[:], in_=whh_mov[:])
            wih_sb = cpool.tile([128, 2048], BF16, tag="wih")
            nc.sync.dma_start(out=wih_sb[:], in_=wih_mov[:])
            brow_sb = cpool.tile([1, 1152], BF16, tag="brow")
            nc.sync.dma_start(out=brow_sb[:], in_=brow[:])
            v_sb = cpool.tile([128, 8], BF16, tag="v")
            nc.sync.dma_start(out=v_sb[:], in_=v_mov[:])
            id_sb = cpool.tile([128, 128], F32, tag="ident")
            nc.sync.dma_start(out=id_sb[:], in_=ident[:])
            m_sb = cpool.tile([64, TC], F32, tag="mask")
            nc.sync.dma_start(out=m_sb[:], in_=m_mask[:])

            # Sacrificial PE instructions: absorb each input-DMA semaphore on
            # PE before any real matmul reads these tensors (Matmult can carry
            # only ONE sync wait; without these, a matmul could need 2+).
            sac = mpsum.tile([128, 128], F32, tag="hT_ps")
            nc.tensor.transpose(sac[:], id_sb[:], id_sb[:])
            sac = mpsum.tile([128, 128], F32, tag="hT_ps")
            nc.tensor.matmul(sac[0:64, 0:64], x_sb[:, 0:64], x_sb[:, 0:64])
            sac = mpsum.tile([128, 128], F32, tag="hT_ps")
            nc.tensor.matmul(sac[0:64, 0:64], whh_sb[:, 0:64], whh_sb[:, 0:64])
            sac = mpsum.tile([128, 128], F32, tag="hT_ps")
            nc.tensor.matmul(sac[0:64, 0:64], wih_sb[:, 0:64], wih_sb[:, 0:64])
            sac = mpsum.tile([128, 128], F32, tag="hT_ps")
            nc.tensor.matmul(sac[0:8, 0:8], v_sb[:, 0:8], v_sb[:, 0:8])
            sac = mpsum.tile([128, 128], F32, tag="hT_ps")
            nc.tensor.matmul(sac[0:128, 0:128], brow_sb[:, 0:128], brow_sb[:, 0:128])

            # ---- alpha phase: logits -> softmax -> alpha_sb [128, TC*4] ----
            alpha_sb = apool.tile([128, TC * 4], F32, tag="alpha")
            exps = apool.tile([64, TC * 4], F32, tag="exps")
            for s0 in range(0, TC, 64):
                sl = min(64, TC - s0)
                lg = mpsum.tile([64, 4 * sl], F32, tag="lg")
                for tt in range(sl):
                    t = s0 + tt
                    for c in range(2):
                        nc.tensor.matmul(
                            lg[:, 4 * tt:4 * tt + 4],
                            x_sb[:, t * 128 + 64 * c: t * 128 + 64 * c + 64],
                            v_sb[:, 4 * c:4 * c + 4],
                            start=(c == 0), stop=(c == 1),
                        )
                nc.scalar.activation(
                    exps[:, s0 * 4:(s0 + sl) * 4], lg[:],
                    mybir.ActivationFunctionType.Exp,
                )
                ssum = wpool.tile([64, sl], F32, tag="ssum")
                e3 = exps[:, s0 * 4:(s0 + sl) * 4].rearrange("p (t k) -> p t k", k=4)
                nc.vector.tensor_reduce(
                    ssum[:], e3, mybir.AxisListType.X, mybir.AluOpType.add
                )
                srec = wpool.tile([64, sl], F32, tag="srec")
                nc.vector.reciprocal(srec[:], ssum[:])
                nc.vector.tensor_tensor(
                    srec[:], srec[:], m_sb[:, s0:s0 + sl], mybir.AluOpType.mult
                )
                a3 = alpha_sb[:64, s0 * 4:(s0 + sl) * 4].rearrange(
                    "p (t k) -> p t k", k=4)
                sr3 = srec[:].unsqueeze(2)
                for k in range(4):
                    nc.vector.tensor_tensor(
                        a3[:, :, k:k + 1], e3[:, :, k:k + 1], sr3,
                        mybir.AluOpType.mult,
                    )
            # replicate alpha rows 0:64 -> 64:128
            nc.sync.dma_start(out=alpha_sb[64:128, :], in_=alpha_sb[0:64, :])
            # DVE touch: observe the replicate-DMA completion so combine ops
            # never need a second (DMA) wait.
            adummy = wpool.tile([64, 1], F32, tag="adummy")
            nc.vector.tensor_copy(adummy[:], alpha_sb[64:128, 0:1])

            # ---- scan ----
            h_prev = spool.tile([128, 128], BF16, tag="hT")
            nc.vector.memset(h_prev[:], 0.0)
            y_stage = ypool.tile([128, YB * 128], F32, tag="ystage")

            cell_tiles = {}

            def emit_xw(t):
                """bias + x.Wih for step t into a fresh PSUM tile."""
                cp = cells_pool.tile([128, 512], F32, tag="cells")
                cell_tiles[t] = cp
                for f in range(2):
                    nc.tensor.matmul(
                        cp[64 * f:64 * f + 64, :],
                        brow_sb[:, 64 * f:64 * f + 64],
                        brow_sb[:, 128 + 512 * f:128 + 512 * f + 512],
                        start=True, stop=False,
                        tile_position=(0, 64 * f),
                        skip_group_check=True,
                    )
                    for c in range(2):
                        nc.tensor.matmul(
                            cp[64 * f:64 * f + 64, :],
                            x_sb[:, t * 128 + 64 * c: t * 128 + 64 * c + 64],
                            wih_sb[:, 1024 * c + 512 * f: 1024 * c + 512 * f + 512],
                            start=False, stop=False,
                            tile_position=(0, 64 * f),
                            skip_group_check=True,
                        )

            for t in range(PIPE):
                emit_xw(t)

            for t in range(TC):
                if t + PIPE < TC:
                    emit_xw(t + PIPE)
                cp = cell_tiles.pop(t)
                # recurrent matmuls accumulate h.Whh into the cell tile
                for f in range(2):
                    for c in range(2):
                        nc.tensor.matmul(
                            cp[64 * f:64 * f + 64, :],
                            h_prev[:, 64 * c:64 * c + 64],
                            whh_sb[:, 1024 * c + 512 * f: 1024 * c + 512 * f + 512],
                            start=False, stop=(c == 1),
                            tile_position=(0, 64 * f),
                            skip_group_check=True,
                        )
                # tanh in place (PSUM -> PSUM): keeps the cells tile single-
                # foreign-writer so downstream waits stay within budget
                nc.scalar.activation(cp[:], cp[:], mybir.ActivationFunctionType.Tanh)
                h_new = wpool.tile([128, 128], F32, tag="hnew")
                nc.vector.tensor_scalar(
                    h_new[:], cp[:, 0:128], alpha_sb[:, 4 * t:4 * t + 1], None,
                    mybir.AluOpType.mult,
                )
                for k in range(1, 4):
                    nc.vector.scalar_tensor_tensor(
                        h_new[:], cp[:, 128 * k:128 * k + 128],
                        alpha_sb[:, 4 * t + k:4 * t + k + 1], h_new[:],
                        mybir.AluOpType.mult, mybir.AluOpType.add,
                    )
                hT_ps = mpsum.tile([128, 128], F32, tag="hT_ps")
                nc.tensor.transpose(hT_ps[:], h_new[:], id_sb[:])
                h_prev = spool.tile([128, 128], BF16, tag="hT")
                nc.vector.tensor_copy(h_prev[:], hT_ps[:])
                if t >= W:
                    tl = (t - W) % YB
                    nc.vector.tensor_copy(
                        y_stage[:, tl * 128:tl * 128 + 128], h_new[:])
                    if tl == YB - 1:
                        b0 = (t - W) - (YB - 1)
                        nc.sync.dma_start(
                            out=y_out[:, b0 * 128:(b0 + YB) * 128], in_=y_stage[:])
                        y_stage = ypool.tile([128, YB * 128], F32, tag="ystage")

    _split_multi_waits(nc)
    return nc


_NO_SPLIT = ("InstEventSemaphore", "InstNoOp")


def _split_multi_waits(nc):
    """Walrus codegen allows only one sync-wait per compute instruction.

    Tile emits 2-3 on PSUM-reuse joins; move the extras onto same-engine
    NOPs inserted just before (engine streams are in-order, so waiting on
    the preceding NOP is equivalent)."""
    nsplit = 0
    for fn in nc.m.functions:
        for bb in fn.blocks:
            out = []
            for ins in bb.instructions:
                si = ins.sync_info
                waits = list(si.on_wait or []) if si is not None else []
                if len(waits) > 1 and type(ins).__name__ not in _NO_SPLIT:
                    for j, w in enumerate(waits[:-1]):
                        nop = mybir.InstNoOp(
                            name=f"{ins.name}-wsplit{j}", engine=ins.engine)
                        nop.sync_info = mybir.SyncInfo(on_wait=[w], on_update=[])
                        out.append(nop)
                        nsplit += 1
                    si.on_wait = [waits[-1]]
                out.append(ins)
            bb.instructions = out
    return nsplit


_NC_CACHE = None


def _get_nc():
    global _NC_CACHE
    if _NC_CACHE is None:
        _NC_CACHE = build_bass()
    return _NC_CACHE


def kernel(x, temperature, Wih, Whh, b_ih, b_hh, V):
    x = np.asarray(x, np.float32)
    Wih = np.asarray(Wih, np.float32)
    Whh = np.asarray(Whh, np.float32)
    bias = np.asarray(b_ih, np.float32) + np.asarray(b_hh, np.float32)
    V = np.asarray(V, np.float32) / float(np.float32(temperature))
    bf = ml_dtypes.bfloat16

    # moving operands: col = 1024*c + 512*f + 128*k + g''  -> W[k, 128f+g'', 128c+p]
    def mov(wt):  # wt [K, H(out g), 256(in)]
        m = np.zeros((128, 2048), np.float32)
        for c in range(2):
            for f in range(2):
                for k in range(4):
                    blk = wt[k, 128 * f:128 * f + 128, 128 * c:128 * c + 128]
                    m[:, 1024 * c + 512 * f + 128 * k: 1024 * c + 512 * f + 128 * (k + 1)] = blk.T
        return m.astype(bf)

    whh_m = mov(Whh)
    wih_m = mov(np.transpose(Wih, (0, 1, 2)))  # Wih already [K, H, E]

    brow_r = np.zeros((1, 1152), np.float32)
    brow_r[0, :128] = 1.0
    for f in range(2):
        for k in range(4):
            brow_r[0, 128 + 512 * f + 128 * k:128 + 512 * f + 128 * (k + 1)] = \
                bias[k, 128 * f:128 * f + 128]
    brow_r = brow_r.astype(bf)
    v_m = np.zeros((128, 8), np.float32)
    for c in range(2):
        v_m[:, 4 * c:4 * c + 4] = V[:, 128 * c:128 * c + 128].T
    v_m = v_m.astype(bf)
    ident = np.eye(128, dtype=np.float32)

    in_maps = []
    for core in range(NCORES):
        t0 = TCHUNK * core - W
        xc = np.zeros((TC, 128, 128), np.float32)  # [t, p, 64c+b]
        lo = max(t0, 0)
        # x[b, t, 128c+p] -> xc[t-t0, p, 64c+b]
        xs = x[:, lo:t0 + TC]  # [B, n, E]
        n = xs.shape[1]
        xr = xs.reshape(B, n, 2, 128).transpose(1, 3, 2, 0)  # [n, p, c, b]
        xc[TC - n:] = xr.reshape(n, 128, 128)
        m = np.ones((64, TC), np.float32)
        if core == 0:
            m[:, :W] = 0.0
        in_maps.append({
            "x_t": np.ascontiguousarray(
                xc.transpose(1, 0, 2).reshape(128, TC * 128)).astype(bf),
            "whh_mov": whh_m, "wih_mov": wih_m, "brow": brow_r,
            "v_mov": v_m, "ident": ident,
            "m_mask": m,
        })

    nc = _get_nc()
    kwargs = {}
    if os.environ.get("BENCH_TRACE"):
        kwargs = dict(trace=True, tmpdir=os.environ.get("BENCH_TMPDIR") or None)
    res = run_bass_kernel_spmd(nc, in_maps, core_ids=list(range(NCORES)), **kwargs)
    global LAST_RESULTS
    LAST_RESULTS = res

    y = np.zeros((B, T, H), np.float32)
    for core in range(NCORES):
        yo = res.results[core]["y_out"]  # [128, TCHUNK*128]: [(f,b), (t, g'')]
        yo = yo.reshape(2, 64, TCHUNK, 128)  # [f, b, t, g'']
        y[:, TCHUNK * core:TCHUNK * (core + 1)] = (
            yo.transpose(1, 2, 0, 3).reshape(64, TCHUNK, 256))
    h_last = y[:, -1].copy()
    return y, h_last



